# revision 21
# baseline (speedup 1.0000x reference)
"""AttentionHead kernel for 8x TRN2 NeuronCores (Bass/Tile on Bacc).

Problem: single-head attention, S=4096, B=4, D=128, C=K=V=64, f32 inputs,
int32 {0,1} mask [1, S, S] applied before softmax (mask==0 -> -inf).

Sharding: queries sharded across 8 cores (512 q/core, all 4 batches per
core). Host passes everything pre-laid-out in bf16: key/query/value
feature-major [D, B, S*], mask slice pre-transposed [S, QS]. Per-core
HBM read ~12.6 MiB.

Math (per core, per batch), all PE contractions on partitions:
  k_proj = wk @ key              (NO bias: softmax over s is invariant
                                  to the per-q offset bk.(q+bq))
  q_proj = wq @ query + bq
  v_projb[s,c] = value[s,:] @ wv[c,:] + bv[c]     (projected UP FRONT)
  v_ext[s, 0:64] = v_projb;  v_ext[s, 64] = 1     (ones column)
  scores^T[s, q] = sum_c k_proj[s,c] q_proj[q,c]  (even/odd row-split)
  alpha = exp(scores^T / 8) * maskT               (ACT exp, DVE mult)
  comb[c', q] = sum_s v_ext[s,c'] alpha[s,q]      (M=65 matmul: row 64
                                                   accumulates the softmax
                                                   denominator for free --
                                                   no separate sums matmuls)
  out[q, :] = (comb[0:64]/comb[64]).T             (PE transpose + scale;
                                                   the bv*sums term divides
                                                   out to exactly +bv)

v_ext construction: v_projT [c, s] via wvT-stationary matmuls (c on
partitions), bias added during the PSUM->SBUF copy (per-partition bias),
ones as literal row 64 of an [80, S] staging tile, then HWDGE xbar
DMA-transpose [80, 512] -> [128, 4, 80] slots (contiguous destination;
cols 65..79 of each slot are dead padding). va2 lhsT = v_ext[:, 80t:80t+65].

Perf structure:
  - staged software pipeline: iteration u issues scores(u), exp+mask(u-1),
    va2(u-2). PE never waits on the scores->exp->mask->alpha chain, so it
    stays continuously busy (required to reach/hold the high PE p-state).
  - one full-pair exp [128,1024] per iteration (per-instruction overhead
    dominates small ACT/DVE ops; bigger is better).
  - ACT: exp + projection-copy/bias; DVE: mask mult + epilogue; PE: 3.2
    matmul issues per pair instead of 5 (sums matmuls eliminated).
  - host-side bf16 removes all f32->bf16 CAST traffic and halves DMA.
  - next batch's key/query/value DMAs are emitted before this batch's
    main loop so the transfers hide under compute.
"""

import os
import sys

import numpy as np

if "/opt/trn_rl_repo" not in sys.path:
    sys.path.insert(0, "/opt/trn_rl_repo")

S, B, D, C = 4096, 4, 128, 64
NCORES = 8
QS = S // NCORES  # 512 queries per core
QT = QS // 128  # 4 q tiles
ST = S // 128  # 32 s tiles
NP = ST // 2  # 16 even/odd s-tile pairs
KEXP = 1  # exp/mask lag behind scores (pairs)
KVA = 4  # va lag behind scores (pairs): generous slack so va2
         # never races the exp/mask chain or the v_ext transposes
SLOT = 80  # v_ext slot width (64 proj + 1 ones + 15 pad; 160B = 32B-aligned)
SCALE = 0.125  # 1/sqrt(64)

LAST_RESULT = None
KVER = 30  # bumped per kernel revision: defeats HLO-fingerprint NEFF-cache aliasing


def _install_ntff_hook():
    """The grading/axon image lacks antenv.axon_hooks; recreate it so
    trace=True can capture NTFF profiles. Harmless no-op when unavailable."""
    import types

    try:
        import antenv

        try:
            from antenv import axon_hooks  # noqa: F401

            return
        except ImportError:
            pass
        from trn_agent_boot.trn_boot import _ntff_profile_via_ctypes

        mod = types.ModuleType("antenv.axon_hooks")
        _h = [_ntff_profile_via_ctypes("/opt/axon/libaxon_pjrt.so")]
        mod.get_axon_ntff_profile_hook = lambda: _h[0]
        mod.set_axon_ntff_profile_hook = lambda h: _h.__setitem__(0, h)
        sys.modules["antenv.axon_hooks"] = mod
        antenv.axon_hooks = mod
    except Exception:
        pass


def _build_nc():
    import concourse.mybir as mybir
    from concourse import bacc
    from concourse.masks import make_identity
    from concourse.tile import TileContext

    f32 = mybir.dt.float32
    bf16 = mybir.dt.bfloat16
    AF = mybir.ActivationFunctionType

    nc = bacc.Bacc("TRN2")

    key_d = nc.dram_tensor("key", [D, B, S], bf16, kind="ExternalInput")
    query_d = nc.dram_tensor("query", [D, B, QS], bf16, kind="ExternalInput")
    value_d = nc.dram_tensor("value", [D, B, S], bf16, kind="ExternalInput")
    mask_d = nc.dram_tensor("mask", [S, QS], bf16, kind="ExternalInput")
    wkT_d = nc.dram_tensor("wkT", [D, C], bf16, kind="ExternalInput")
    wqT_d = nc.dram_tensor("wqT", [D, C], bf16, kind="ExternalInput")
    wvT_d = nc.dram_tensor("wvT", [D, SLOT], bf16, kind="ExternalInput")
    bq2_d = nc.dram_tensor("bq2", [128, 1], f32, kind="ExternalInput")
    bvx_d = nc.dram_tensor("bvx", [SLOT, 1], f32, kind="ExternalInput")
    out_d = nc.dram_tensor("out", [QS, B, C], f32, kind="ExternalOutput")
    # dummy input whose shape encodes the kernel revision: the PJRT-side NEFF
    # cache keys on the HLO signature (not the embedded BIR), so same-shaped
    # kernel revisions would otherwise silently alias to a stale executable.
    nc.dram_tensor("vtag", [KVER], f32, kind="ExternalInput")

    with TileContext(nc) as tc:
        with (
            tc.tile_pool(name="consts", bufs=1) as consts,
            tc.tile_pool(name="big", bufs=1) as big,
            tc.tile_pool(name="pb", bufs=2) as pb,
            tc.tile_pool(name="work", bufs=4) as work,
            tc.tile_pool(name="expool", bufs=3) as expool,
            tc.tile_pool(name="apool", bufs=6) as apool,
            tc.tile_pool(name="scps", bufs=3, space="PSUM") as scps,
            tc.tile_pool(name="ppps", bufs=1, space="PSUM") as ppps,
            tc.tile_pool(name="accps", bufs=1, space="PSUM") as accps,
        ):
            # ---------------- constants ----------------
            ident_f = consts.tile([128, 128], f32, tag="ident_f")
            make_identity(nc, ident_f[:])

            # weights arrive pre-transposed bf16 from the host; wvT is
            # pre-padded to [D, 80] (zero cols 64-79) and bvx pre-extended
            # (row 64 = 1.0 -> the ones row of v_projbT, rows 65-79 = 0).
            wT = {}
            for name, d_t, cols in (("k", wkT_d, C), ("q", wqT_d, C), ("v", wvT_d, SLOT)):
                wt_sb = consts.tile([D, cols], bf16, name=f"wt_sb_{name}")
                nc.sync.dma_start(out=wt_sb[:], in_=d_t[:, :])
                wT[name] = wt_sb
            bq2 = consts.tile([128, 1], f32, tag="bq2")
            nc.sync.dma_start(out=bq2[:], in_=bq2_d[:, :])
            bv1x = consts.tile([SLOT, 1], f32, tag="bv1x")
            nc.sync.dma_start(out=bv1x[:], in_=bvx_d[:, :])

            maskT = big.tile([128, ST * QS], bf16, tag="maskT")

            def load_batch(b):
                keyT = pb.tile([128, S], bf16, tag="keyT")
                qT = pb.tile([128, QS], bf16, tag="qT")
                nc.sync.dma_start(out=qT[:], in_=query_d[:, b, :])
                for h in range(2):
                    nc.sync.dma_start(
                        out=keyT[:, h * 2048 : (h + 1) * 2048],
                        in_=key_d[:, b, h * 2048 : (h + 1) * 2048],
                    )
                valT = pb.tile([128, S], bf16, tag="valT")
                for h in range(2):
                    nc.sync.dma_start(
                        out=valT[:, h * 2048 : (h + 1) * 2048],
                        in_=value_d[:, b, h * 2048 : (h + 1) * 2048],
                    )
                return keyT, qT, valT

            def proj_tiles():
                k_projT2 = pb.tile([128, NP * 128], bf16, tag="k_projT2")
                q_projT3 = pb.tile([128, QS], bf16, tag="q_projT3")
                v_projbT = pb.tile([SLOT, S], bf16, tag="v_projbT")
                v_ext = pb.tile([128, ST * SLOT], bf16, tag="v_ext")
                return k_projT2, q_projT3, v_ext, v_projbT

            def kq_tasks(tiles, keyT, qT):
                """Score-side projection (mm, copy) phase pairs."""
                k_projT2, q_projT3, v_ext, v_projbT = tiles
                keyT_v = keyT[:].rearrange(
                    "d (c bb two j) -> d c bb two j", c=4, bb=4, two=2
                )
                pairs = []

                def qp_mm(cell={}):
                    qp_ps = ppps.tile([128, 512], f32, tag="pp", name="qp_ps")
                    nc.tensor.matmul(
                        qp_ps[:64, :], wT["q"][:], qT[:], start=True, stop=True
                    )
                    nc.tensor.matmul(
                        qp_ps[64:, :],
                        wT["q"][:],
                        qT[:],
                        start=True,
                        stop=True,
                        tile_position=(0, 64),
                    )
                    cell["ps"] = qp_ps
                    qp_mm.cell = cell

                def qp_cp():
                    nc.vector.tensor_scalar_add(
                        out=q_projT3[:], in0=qp_mm.cell["ps"], scalar1=bq2[:]
                    )

                pairs.append((qp_mm, qp_cp))
                for g in range(4):
                    cell = {}

                    def kp_mm(g=g, cell=cell):
                        kp_ps = ppps.tile([128, 512], f32, tag="pp", name="kp_ps")
                        nc.tensor.matmul(
                            kp_ps[:64, :],
                            wT["k"][:],
                            keyT_v[:, g, :, 0, :],
                            start=True,
                            stop=True,
                        )
                        nc.tensor.matmul(
                            kp_ps[64:, :],
                            wT["k"][:],
                            keyT_v[:, g, :, 1, :],
                            start=True,
                            stop=True,
                            tile_position=(0, 64),
                        )
                        cell["ps"] = kp_ps

                    def kp_cp(g=g, cell=cell):
                        dst = k_projT2[:, g * 512 : (g + 1) * 512]
                        if g == 0:
                            nc.scalar.copy(out=dst, in_=cell["ps"][:])
                        else:
                            nc.vector.tensor_copy(out=dst, in_=cell["ps"][:])

                    pairs.append((kp_mm, kp_cp))
                return pairs

            def v_tasks(tiles, valT):
                """Value-side (mm, copy) phase pairs + transpose steps."""
                k_projT2, q_projT3, v_ext, v_projbT = tiles
                pairs = []
                for i in range(8):
                    cell = {}

                    def vp_mm(i=i, cell=cell):
                        vp_ps = ppps.tile([SLOT, 512], f32, tag="pp", name="vp_ps")
                        nc.tensor.matmul(
                            vp_ps[:],
                            wT["v"][:],
                            valT[:, i * 512 : (i + 1) * 512],
                            start=True,
                            stop=True,
                        )
                        cell["ps"] = vp_ps

                    def vp_cp(i=i, cell=cell):
                        dst = v_projbT[:, i * 512 : (i + 1) * 512]
                        if i % 4 == 0:
                            nc.scalar.activation(
                                out=dst,
                                in_=cell["ps"][:],
                                func=AF.Identity,
                                bias=bv1x[:],
                                scale=1.0,
                            )
                        else:
                            nc.vector.tensor_scalar_add(
                                out=dst, in0=cell["ps"][:], scalar1=bv1x[:]
                            )

                    pairs.append((vp_mm, vp_cp))
                    if i % 2 == 1:

                        def tr_task(i=i):
                            j = i // 2
                            nc.sync.dma_start_transpose(
                                out=v_ext[
                                    :, j * 8 * SLOT : (j + 1) * 8 * SLOT
                                ].rearrange("p (tt c) -> p tt c", c=SLOT),
                                in_=v_projbT[:, j * 1024 : (j + 1) * 1024],
                            )

                        pairs.append((tr_task, None))
                return pairs

            def stagger(pairs):
                """Turn (mm, copy) pairs into per-iteration steps where each
                step emits the PREVIOUS task's copy before this task's mm, so
                the single-buffer pp ring never stalls the PE in-order queue."""
                steps = []
                prev_cp = [None]

                def mk(mm, pc):
                    def step():
                        if pc is not None:
                            pc()
                        mm()

                    return step

                for mm, cp in pairs:
                    steps.append(mk(mm, prev_cp[0]))
                    prev_cp[0] = cp
                if prev_cp[0] is not None:
                    steps.append(lambda pc=prev_cp[0]: pc())
                return steps

            def epilogue_tasks(b, va2_ps):
                """Deferred epilogue steps, drained inside the NEXT batch's
                loop so the batch boundary never idles PE/ACT (idle gaps
                re-throttle the PE clock). Step 0 (comb copy) frees va2_ps
                and must run early; the rest can trail."""
                cell = {}
                steps = []

                def comb_step():
                    comb = work.tile([C + 1, QS], f32, tag="comb")
                    nc.vector.tensor_copy(out=comb[:], in_=va2_ps[:])
                    fin = work.tile([128, QT * C], f32, tag="fin")
                    cell["comb"] = comb
                    cell["fin"] = fin

                steps.append(comb_step)
                for qt in range(QT):

                    def qt_step(qt=qt):
                        ot_ps = ppps.tile([128, C + 1], f32, tag="pp", name="ot_ps")
                        nc.tensor.transpose(
                            ot_ps[:],
                            cell["comb"][:, qt * 128 : (qt + 1) * 128],
                            ident_f[: C + 1, : C + 1],
                        )
                        recip = work.tile([128, 1], f32, tag="recip")
                        nc.vector.reciprocal(recip[:], ot_ps[:, C : C + 1])
                        nc.vector.tensor_scalar_mul(
                            out=cell["fin"][:, qt * C : (qt + 1) * C],
                            in0=ot_ps[:, :C],
                            scalar1=recip[:],
                        )

                    steps.append(qt_step)

                def out_step():
                    nc.sync.dma_start(
                        out=out_d[:, b, :].rearrange("(qt p) c -> p qt c", p=128),
                        in_=cell["fin"][:].rearrange("p (qt c) -> p qt c", c=C),
                    )

                steps.append(out_step)
                return steps

            def main_loop(b, tiles, tasks):
                """Pipelined pair loop; drains `tasks` (adjacent batches'
                projection/epilogue work) so batch boundaries never idle
                an engine."""
                k_projT2, q_projT3, v_ext, v_projbT = tiles
                va2_ps = accps.tile([C + 1, QS], f32, tag="va", name="va2")
                scs = {}
                alphas = {}
                for u in range(NP + KVA):
                    if u < NP:
                        sc = scps.tile([128, 1024], f32, tag="sc", name="sc")
                        nc.tensor.matmul(
                            sc[:, :512],
                            k_projT2[:64, u * 128 : (u + 1) * 128],
                            q_projT3[:64, :],
                            start=True,
                            stop=True,
                        )
                        nc.tensor.matmul(
                            sc[:, 512:],
                            k_projT2[64:, u * 128 : (u + 1) * 128],
                            q_projT3[64:, :],
                            start=True,
                            stop=True,
                        )
                        scs[u] = sc
                    if KEXP <= u < NP + KEXP:
                        v = u - KEXP
                        sc = scs.pop(v)
                        ex = expool.tile([128, 1024], bf16, tag="ex")
                        nc.scalar.activation(
                            out=ex[:], in_=sc[:], func=AF.Exp, scale=SCALE
                        )
                        alpha = apool.tile([128, 1024], bf16, tag="alpha")
                        nc.vector.tensor_mul(
                            alpha[:], ex[:], maskT[:, v * 1024 : (v + 1) * 1024]
                        )
                        alphas[v] = alpha
                    if u >= KVA:
                        v = u - KVA
                        alpha = alphas.pop(v)
                        for h in range(2):
                            st = 2 * v + h
                            nc.tensor.matmul(
                                va2_ps[:],
                                v_ext[:, st * SLOT : st * SLOT + C + 1],
                                alpha[:, h * 512 : (h + 1) * 512],
                                start=(st == 0),
                                stop=(st == ST - 1),
                            )
                    if tasks:
                        tasks.pop(0)()
                    if len(tasks) > 10:
                        tasks.pop(0)()
                return va2_ps

            # ---------------- batch pipeline ----------------
            loaded = load_batch(0)
            tiles = proj_tiles()
            for mm, cp in kq_tasks(tiles, loaded[0], loaded[1]):
                mm()
                if cp is not None:
                    cp()
            pending = stagger(v_tasks(tiles, loaded[2]))
            for j in range(8):
                nc.sync.dma_start(
                    out=maskT[:, j * 2048 : (j + 1) * 2048].rearrange(
                        "p (t q) -> p t q", t=4
                    ),
                    in_=mask_d[j * 512 : (j + 1) * 512, :].rearrange(
                        "(t p) q -> p t q", p=128
                    ),
                )
            for b in range(B):
                if b + 1 < B:
                    nxt_loaded = load_batch(b + 1)
                    nxt_tiles = proj_tiles()
                    pending += stagger(
                        kq_tasks(nxt_tiles, nxt_loaded[0], nxt_loaded[1])
                        + v_tasks(nxt_tiles, nxt_loaded[2])
                    )
                va2_ps = main_loop(b, tiles, pending)
                ep = epilogue_tasks(b, va2_ps)
                pending[:0] = [ep[0]]
                pending.extend(ep[1:])
                if b + 1 < B:
                    tiles = nxt_tiles
            while pending:
                pending.pop(0)()

    nc.finalize()
    return nc


_nc_cache = None


def kernel(**inputs):
    global _nc_cache, LAST_RESULT
    _install_ntff_hook()
    import ml_dtypes

    from concourse.bass_utils import run_bass_kernel_spmd

    bf16 = ml_dtypes.bfloat16
    arrs = {k: np.asarray(v) for k, v in inputs.items()}
    key = np.ascontiguousarray(
        arrs["key"].astype(np.float32).transpose(2, 1, 0)
    ).astype(bf16)
    value = np.ascontiguousarray(
        arrs["value"].astype(np.float32).transpose(2, 1, 0)
    ).astype(bf16)
    query = np.ascontiguousarray(arrs["query"], dtype=np.float32)
    mask = np.ascontiguousarray(arrs["mask"], dtype=np.int32)
    if mask.ndim == 3:
        mask = mask[0]

    wkT = np.ascontiguousarray(arrs["wk_w"].astype(np.float32).T).astype(bf16)
    wqT = np.ascontiguousarray(arrs["wq_w"].astype(np.float32).T).astype(bf16)
    wvT = np.zeros([D, SLOT], dtype=bf16)
    wvT[:, :C] = arrs["wv_w"].astype(np.float32).T.astype(bf16)
    bq2h = np.zeros([128, 1], np.float32)
    bq2h[:C, 0] = arrs["wq_b"].astype(np.float32)
    bq2h[C:, 0] = arrs["wq_b"].astype(np.float32)
    bvx = np.zeros([SLOT, 1], np.float32)
    bvx[:C, 0] = arrs["wv_b"].astype(np.float32)
    bvx[C, 0] = 1.0

    if _nc_cache is None:
        _nc_cache = _build_nc()
    nc = _nc_cache

    in_maps = []
    for i in range(NCORES):
        q0 = i * QS
        in_maps.append(
            {
                "key": key,
                "value": value,
                "query": np.ascontiguousarray(
                    query[q0 : q0 + QS].transpose(2, 1, 0)
                ).astype(bf16),
                "mask": np.ascontiguousarray(mask[q0 : q0 + QS].T).astype(bf16),
                "wkT": wkT,
                "wqT": wqT,
                "wvT": wvT,
                "bq2": bq2h,
                "bvx": bvx,
                "vtag": np.zeros([KVER], np.float32),
            }
        )

    trace = bool(int(os.environ.get("KERNEL_TRACE", "0")))
    kw = {}
    if trace:
        kw = dict(trace=True, trace_cores=[0])
    try:
        res = run_bass_kernel_spmd(nc, in_maps, core_ids=list(range(NCORES)), **kw)
    except Exception:
        # transient device wedge (e.g. NRT_EXEC_UNIT_UNRECOVERABLE from an
        # earlier crashed process): one retry after the runtime re-opens
        res = run_bass_kernel_spmd(nc, in_maps, core_ids=list(range(NCORES)), **kw)
    LAST_RESULT = res
    out = np.concatenate([r["out"] for r in res.results], axis=0)
    return out
